# revision 1
# baseline (speedup 1.0000x reference)
"""AttentionCritic Bass/Tile kernel for TRN2.

Math (per core, shard of B):
  s_enc  = relu(states @ Ws + bs)              [A,BL,H]
  sa_enc = relu([states,actions] @ We + be)    [A,BL,H]
  keys   = sa_enc @ Wk[n]   sel = s_enc @ Wsel[n]   vals = relu(sa_enc @ Wv[n] + bv)
  logits[n,i,j,b] = sel[n,i,b,:].keys[n,j,b,:] / sqrt(D), j != i
  w = softmax_j(logits);  other = sum_j w * vals
  h = relu([sa_enc, other_cat] @ W1 + b1);  q = h @ W2 + b2

Layouts (SBUF):
  agent i = 2*c + q  (q = i%2 "parity half" on partitions, c = i//2 in free)
  slabs [(q,n,d)=128 or (q,h)=128, (c=4, b=NB)]  free F = 4*NB
  rotation j = i + r (mod 8) becomes a flat free-offset by (r//2)*NB into
  KEY/VAL (even r) or the pre-swapped KSW/VSW slabs (odd r).
  logits/softmax live compact on [(r,q,n)=56, (c,b)] ; softmax weights are
  broadcast to [(q,n,d)=128, ...] by a replicating SBUF->SBUF DMA.
"""

import numpy as np
import ml_dtypes

from concourse import bass, bacc, tile, mybir
from contextlib import ExitStack

A, S, AD, H, NH, D = 8, 128, 16, 64, 4, 16
F32 = mybir.dt.float32
BF16 = mybir.dt.bfloat16
AF = mybir.ActivationFunctionType
ALU = mybir.AluOpType

# Wconst column offsets (bf16, [128, NWC])
_off = {}
_cur = 0
for _name, _w in [("enc0", 128), ("enc1", 128), ("ea0", 128), ("ea1", 128),
                  ("sel", 128), ("k", 128), ("ksw", 128), ("v", 128), ("vsw", 128),
                  ("r", 7 * 56), ("zo", 56), ("w1t", 128), ("w1b", 128), ("w2", 2)]:
    _off[_name] = _cur
    _cur += _w
NWC = _cur


def host_prep(states, actions, Ws, bs, We, be, Wk, Wsel, Wv, bv, W1, b1, W2, b2):
    """Full inputs -> (global arrays for sharding, const arrays).

    Returns dict with:
      st  [S, A, B] bf16   (shard along last axis)
      ac  [AD, A, B] bf16
      wc  [128, NWC] bf16
      bc  [128, 8] f32
    """
    bf = ml_dtypes.bfloat16
    states = np.asarray(states, np.float32)
    actions = np.asarray(actions, np.float32)
    st = np.ascontiguousarray(np.transpose(states.astype(bf), (2, 0, 1)))
    ac = np.ascontiguousarray(np.transpose(actions.astype(bf), (2, 0, 1)))

    Ws, We, Wk, Wsel, Wv, W1, W2 = [np.asarray(x, np.float32) for x in (Ws, We, Wk, Wsel, Wv, W1, W2)]
    bs, be, bv, b1, b2 = [np.asarray(x, np.float32) for x in (bs, be, bv, b1, b2)]

    # flat per-head weight blocks [h, n*16+d]
    def flat(Wn):  # [NH, H, D] -> [H, NH*D]
        return np.transpose(Wn, (1, 0, 2)).reshape(H, NH * D)

    wk_f, wsel_f, wv_f = flat(Wk), flat(Wsel), flat(Wv)

    wc = np.zeros((128, NWC), np.float32)
    # encoders
    wc[:, _off["enc0"]:_off["enc0"] + 64] = Ws
    wc[:, _off["enc0"] + 64:_off["enc0"] + 128] = We[:128]
    wc[:, _off["enc1"]:_off["enc1"] + 64] = We[:128]
    wc[:, _off["enc1"] + 64:_off["enc1"] + 128] = Ws
    wc[0:16, _off["ea0"] + 64:_off["ea0"] + 128] = We[128:144]
    wc[0:16, _off["ea1"]:_off["ea1"] + 64] = We[128:144]
    # kvs (pair block structure). SEN is (q=i%2) layout; SAE is inverted (qi=1-i%2).
    for q in (0, 1):
        r0, c0 = q * 64, q * 64
        wc[r0:r0 + 64, _off["sel"] + c0:_off["sel"] + c0 + 64] = wsel_f          # diag
        wc[r0:r0 + 64, _off["k"] + (64 - c0):_off["k"] + (64 - c0) + 64] = wk_f  # anti
        wc[r0:r0 + 64, _off["ksw"] + c0:_off["ksw"] + c0 + 64] = wk_f            # diag
        wc[r0:r0 + 64, _off["v"] + (64 - c0):_off["v"] + (64 - c0) + 64] = wv_f  # anti
        wc[r0:r0 + 64, _off["vsw"] + c0:_off["vsw"] + c0 + 64] = wv_f            # diag
    # d-reduce stationaries R_r [128, 56] (scaled by 1/sqrt(D))
    for r in range(1, 8):
        base = _off["r"] + (r - 1) * 56
        for q in (0, 1):
            for n in range(NH):
                k0 = q * 64 + n * 16
                wc[k0:k0 + 16, base + (r - 1) * 8 + q * 4 + n] = 1.0 / np.sqrt(np.float32(D))
    # Z replicate-ones [56, 56]
    for rp in range(7):
        for r in range(7):
            for qn in range(8):
                wc[rp * 8 + qn, _off["zo"] + r * 8 + qn] = 1.0
    # critic
    for q in (0, 1):
        r0, c0 = q * 64, q * 64
        wc[r0:r0 + 64, _off["w1t"] + (64 - c0):_off["w1t"] + (64 - c0) + 64] = W1[:64]   # anti (SAE)
        wc[r0:r0 + 64, _off["w1b"] + c0:_off["w1b"] + c0 + 64] = W1[64:]                 # diag
        wc[r0:r0 + 64, _off["w2"] + q:_off["w2"] + q + 1] = W2
    bc = np.zeros((128, 8), np.float32)
    bc[0:64, 0] = bs; bc[64:128, 0] = bs
    bc[0:64, 1] = be; bc[64:128, 1] = be
    bc[:, 2] = np.tile(bv.reshape(-1), 2)[:128] if bv.size * 2 == 128 else np.concatenate([bv.reshape(-1), bv.reshape(-1)])
    bc[0:64, 3] = b1; bc[64:128, 3] = b1
    bc[0:2, 4] = b2[0]
    return {"st": st.astype(bf), "ac": ac.astype(bf),
            "wc": wc.astype(bf), "bc": bc.astype(np.float32)}


def build(BL=4096, NB=256, n_cores=8):
    """Build the single-core graph (SPMD-replicated across cores)."""
    F = 4 * NB
    NT = BL // NB
    assert BL % NB == 0 and NB % 2 == 0
    nc = bacc.Bacc("TRN2", target_bir_lowering=False, debug=False,
                   enable_asserts=False, num_devices=n_cores)
    st_d = nc.dram_tensor("st", [S, A, BL], BF16, kind="ExternalInput").ap()
    ac_d = nc.dram_tensor("ac", [AD, A, BL], BF16, kind="ExternalInput").ap()
    wc_d = nc.dram_tensor("wc", [128, NWC], BF16, kind="ExternalInput").ap()
    bc_d = nc.dram_tensor("bc", [128, 8], F32, kind="ExternalInput").ap()
    out_d = nc.dram_tensor("out", [A, BL], F32, kind="ExternalOutput").ap()

    with tile.TileContext(nc) as tc, ExitStack() as ctx:
        pool_c = ctx.enter_context(tc.tile_pool(name="const", bufs=1))
        pool_in = ctx.enter_context(tc.tile_pool(name="inp", bufs=2))
        pool_sl = ctx.enter_context(tc.tile_pool(name="slab", bufs=2))
        pool_sm = ctx.enter_context(tc.tile_pool(name="small", bufs=2))
        pool_wbc = ctx.enter_context(tc.tile_pool(name="wbc", bufs=4))
        pool_pq = ctx.enter_context(tc.tile_pool(name="pq", bufs=3))
        pool_stg = ctx.enter_context(tc.tile_pool(name="stg", bufs=1))
        ps_enc = ctx.enter_context(tc.tile_pool(name="psenc", bufs=1, space="PSUM"))
        ps_kvs = ctx.enter_context(tc.tile_pool(name="pskvs", bufs=1, space="PSUM"))
        ps_att = ctx.enter_context(tc.tile_pool(name="psatt", bufs=1, space="PSUM"))
        ps_h = ctx.enter_context(tc.tile_pool(name="psh", bufs=1, space="PSUM"))
        ps_q = ctx.enter_context(tc.tile_pool(name="psq", bufs=1, space="PSUM"))

        WC = pool_c.tile([128, NWC], BF16)
        BC = pool_c.tile([128, 8], F32)
        nc.sync.dma_start(out=WC[:], in_=wc_d[:])
        nc.sync.dma_start(out=BC[:], in_=bc_d[:])
        QSTG = pool_stg.tile([2, NT * F], F32, tag="qstg")

        def W(name, k=128, m=128):
            return WC[0:k, _off[name]:_off[name] + m]

        NCH = F // 512  # psum-bank chunks per slab row
        for t in range(NT):
            ST = pool_in.tile([128, A * NB], BF16, tag="st")
            AC = pool_in.tile([16, A * NB], BF16, tag="ac")
            nc.sync.dma_start(out=ST[:].rearrange("p (a b) -> p a b", a=A),
                              in_=st_d[:, :, t * NB:(t + 1) * NB])
            nc.sync.dma_start(out=AC[:].rearrange("p (a b) -> p a b", a=A),
                              in_=ac_d[:, :, t * NB:(t + 1) * NB])

            SEN = pool_sl.tile([128, F], BF16, tag="sen")
            SAE = pool_sl.tile([128, F], BF16, tag="sae")
            # --- encoders: groups of 2 same-parity agents ---
            for g in range(4):
                q, cpair = g % 2, (g // 2) * 2  # agents 2*cpair+q, 2*(cpair+1)+q
                P_enc = ps_enc.tile([128, 2 * NB], F32, tag="enc")
                for k in range(2):
                    a = 2 * (cpair + k) + q
                    seg = slice(k * NB, (k + 1) * NB)
                    nc.tensor.matmul(P_enc[:, seg], W("enc%d" % q),
                                     ST[:, a * NB:(a + 1) * NB], start=True, stop=False)
                    nc.tensor.matmul(P_enc[:, seg], W("ea%d" % q, k=16),
                                     AC[0:16, a * NB:(a + 1) * NB], start=False, stop=True)
                dst = slice(cpair * NB, (cpair + 2) * NB)
                if q == 0:  # psum = [s | sa]
                    nc.scalar.activation(SEN[0:64, dst], P_enc[0:64, :], AF.Relu, bias=BC[0:64, 0:1])
                    nc.scalar.activation(SAE[64:128, dst], P_enc[64:128, :], AF.Relu, bias=BC[64:128, 1:2])
                else:       # psum = [sa | s]
                    nc.scalar.activation(SAE[0:64, dst], P_enc[0:64, :], AF.Relu, bias=BC[0:64, 1:2])
                    nc.scalar.activation(SEN[64:128, dst], P_enc[64:128, :], AF.Relu, bias=BC[64:128, 0:1])

            # --- sel/keys/vals slabs ---
            SEL = pool_sl.tile([128, F], BF16, tag="selk")
            KEY = pool_sl.tile([128, F], BF16, tag="key")
            KSW = pool_sl.tile([128, F], BF16, tag="ksw")
            VAL = pool_sl.tile([128, F], BF16, tag="val")
            VSW = pool_sl.tile([128, F], BF16, tag="vsw")
            for name, src, dst, func, bias in (
                    ("sel", SEN, SEL, AF.Copy, None),
                    ("k", SAE, KEY, AF.Copy, None),
                    ("v", SAE, VAL, AF.Relu, BC[:, 2:3]),
            ):
                PS = ps_kvs.tile([128, F], F32, tag="kvs")
                for p in range(4):
                    nc.tensor.matmul(PS[:, p * NB:(p + 1) * NB], W(name),
                                     src[:, p * NB:(p + 1) * NB], start=True, stop=True)
                if bias is None:
                    nc.scalar.activation(dst[:], PS[:], func)
                else:
                    nc.scalar.activation(dst[:], PS[:], func, bias=bias)
            # swapped slabs from KEY/VAL via DVE partition-shift copies:
            # SW[0:64, c] = SRC[64:128, c] ; SW[64:128, c] = SRC[0:64, (c+1)%4]
            for src, dst in ((KEY, KSW), (VAL, VSW)):
                nc.vector.tensor_copy(dst[0:64, :], src[64:128, :])
                nc.vector.tensor_copy(dst[64:128, 0:3 * NB], src[0:64, NB:F])
                nc.vector.tensor_copy(dst[64:128, 3 * NB:F], src[0:64, 0:NB])

            # --- attention: products, d-reduce ---
            PRD = ps_att.tile([56, F], F32, tag="prd")
            for r in range(1, 8):
                s = r // 2
                SRC = KEY if r % 2 == 0 else KSW
                P_r = pool_pq.tile([128, F], BF16, tag="p")
                if s == 0:
                    nc.vector.tensor_mul(P_r[:], SEL[:], SRC[:])
                else:
                    m = F - s * NB
                    nc.vector.tensor_mul(P_r[:, 0:m], SEL[:, 0:m], SRC[:, s * NB:F])
                    nc.vector.tensor_mul(P_r[:, m:F], SEL[:, m:F], SRC[:, 0:s * NB])
                Rw = WC[:, _off["r"] + (r - 1) * 56:_off["r"] + r * 56]
                for ch in range(NCH):
                    sl = slice(ch * 512, (ch + 1) * 512)
                    nc.tensor.matmul(PRD[:, sl], Rw, P_r[:, sl],
                                     start=(r == 1), stop=(r == 7))

            E = pool_sm.tile([56, F], BF16, tag="e")
            nc.scalar.activation(E[:], PRD[:], AF.Exp)
            ZR = ps_att.tile([56, F], F32, tag="prd")
            for ch in range(NCH):
                sl = slice(ch * 512, (ch + 1) * 512)
                nc.tensor.matmul(ZR[:, sl], W("zo", k=56, m=56), E[:, sl], start=True, stop=True)
            RZ = pool_sm.tile([56, F], F32, tag="rz")
            nc.vector.reciprocal_approx_fast(out=RZ[:], in_=ZR[:])
            Wt = pool_sm.tile([56, F], BF16, tag="w")
            nc.vector.tensor_mul(Wt[:], E[:], RZ[:])

            # --- weighted sum + critic ---
            HP = ps_h.tile([128, F], F32, tag="h")
            for ch in range(NCH):
                sl = slice(ch * 512, (ch + 1) * 512)
                nc.tensor.matmul(HP[:, sl], W("w1t"), SAE[:, sl], start=True, stop=False)
            for r in range(1, 8):
                s = r // 2
                SRC = VAL if r % 2 == 0 else VSW
                WB = pool_wbc.tile([128, F], BF16, tag="wbc")
                nc.sync.dma_start(
                    out=WB[:],
                    in_=Wt[(r - 1) * 8:r * 8, :].unsqueeze(1).to_broadcast((8, 16, F)))
                Q_r = pool_pq.tile([128, F], BF16, tag="q")
                if s == 0:
                    nc.vector.tensor_mul(Q_r[:], WB[:], SRC[:])
                else:
                    m = F - s * NB
                    nc.vector.tensor_mul(Q_r[:, 0:m], WB[:, 0:m], SRC[:, s * NB:F])
                    nc.vector.tensor_mul(Q_r[:, m:F], WB[:, m:F], SRC[:, 0:s * NB])
                for ch in range(NCH):
                    sl = slice(ch * 512, (ch + 1) * 512)
                    nc.tensor.matmul(HP[:, sl], W("w1b"), Q_r[:, sl],
                                     start=False, stop=(r == 7))
            HS = pool_sl.tile([128, F], BF16, tag="h")
            nc.scalar.activation(HS[:], HP[:], AF.Relu, bias=BC[:, 3:4])
            for ch in range(NCH):
                sl = slice(ch * 512, (ch + 1) * 512)
                QP = ps_q.tile([2, 512], F32, tag="qp")
                nc.tensor.matmul(QP[:], W("w2", m=2), HS[:, sl], start=True, stop=True)
                nc.scalar.activation(QSTG[:, t * F + ch * 512:t * F + (ch + 1) * 512],
                                     QP[:], AF.Identity, bias=BC[0:2, 4:5])

        # out[a=2c+q, t*NB+b] <- QSTG[q, (t, c, b)]
        nc.sync.dma_start(
            out=out_d.rearrange("(c q) (t b) -> q t c b", q=2, t=NT),
            in_=QSTG[:].rearrange("q (t c b) -> q t c b", t=NT, c=4))
    nc.compile()
    return nc



# ---------------------------------------------------------------------------
# Harness entry point: full inputs in, full output out (8-core SPMD inside).
# ---------------------------------------------------------------------------
N_CORES = 8
B_FULL = 32768
BL = B_FULL // N_CORES
NB = 256

LAST_EXEC_TIME_NS = None
_CACHED_NC = None


def _get_nc():
    global _CACHED_NC
    if _CACHED_NC is None:
        _CACHED_NC = build(BL=BL, NB=NB, n_cores=N_CORES)
    return _CACHED_NC


def kernel(**inputs) -> np.ndarray:
    global LAST_EXEC_TIME_NS
    from concourse.bass_utils import run_bass_kernel_spmd

    prep = host_prep(**{k: np.asarray(v) for k, v in inputs.items()})
    nc = _get_nc()
    in_maps = []
    for i in range(N_CORES):
        sl = slice(i * BL, (i + 1) * BL)
        in_maps.append({
            "st": np.ascontiguousarray(prep["st"][:, :, sl]),
            "ac": np.ascontiguousarray(prep["ac"][:, :, sl]),
            "wc": prep["wc"],
            "bc": prep["bc"],
        })
    core_ids = list(range(N_CORES))
    try:
        res = run_bass_kernel_spmd(nc, in_maps, core_ids, trace=True)
    except Exception:
        res = run_bass_kernel_spmd(nc, in_maps, core_ids, trace=False)
    LAST_EXEC_TIME_NS = res.exec_time_ns
    out = np.concatenate([np.asarray(res.results[i]["out"]) for i in range(N_CORES)],
                         axis=1)
    return out.reshape(A, B_FULL, 1).astype(np.float32)
